# revision 29
# baseline (speedup 1.0000x reference)
"""AttentionFlow kernel for 8 TRN2 NeuronCores (Bass/Tile).

Math (per batch, masks are all-ones by problem spec):
    wx, wy, wxy = w[:D], w[D:2D], w[2D:]
    s[i,j]  = px[i] + qy[j] + sum_d P[i,d]*wxy[d]*Q[j,d] + b
    pq_att  = softmax_j(s);  pq[i,:] = sum_j pq_att[i,j] * Q[j,:]
    qp_sim  = max_j s;       qp_att = softmax_i(qp_sim)
    qp[:]   = sum_i qp_att[i] * P[i,:]   (tiled over Lp on host)

Device formulation (per core: BC=4 batches, data parallel over B):
    qt_aug[d,j] = wxy[d]*q[j,d] + wx[d]      (host; folds px into the S matmul:
                                              sum_d qt_aug[d,j]*pT[d,i] = s - qy[j] - b)
    S^T = qt_aug^T @ pT                       [j,i] in PSUM (bf16)
    e   = exp(S^T + qyb[j])                   (ACT, per-partition bias qyb = qy + b, host-made)
    u   = max_j e                             (PE 128x128 transposes + DVE free-dim reduce)
    r   = sum_j e                             (DVE free-dim reduce on the same en3)
    Y^T[d,i] = q_nat[:,d]^T @ e               (bf16 PE; copied out unnormalized in bf16)
Host post (cheap, ~0.2% of FLOPs): pq = (Y^T / r).T, qp = (u/sum u) @ P,
tiled broadcast. Softmax max-subtraction is skipped (|s| <= ~7, exp safe in
f32); ratios are mathematically identical to the reference.

Host prep: batch shards 4-per-core; bf16 casts; ALL device inputs are laid
out partition-major on the host so each DMA descriptor is one contiguous
multi-KB line per partition (small descriptors were the main startup cost).
Batch 0's operands go first on the fast HWDGE queues.
"""

import numpy as np
import ml_dtypes

import concourse.bass as bass
import concourse.mybir as mybir
import concourse.tile as tile
from concourse import bacc
from concourse.bass_utils import run_bass_kernel_spmd
from concourse.masks import make_identity

BF16 = mybir.dt.bfloat16
F32 = mybir.dt.float32
AF = mybir.ActivationFunctionType

B, LP, LQ, D = 32, 1024, 128, 256
NCORES = 8
BC = B // NCORES        # batches per core
NK = D // 128           # d-chunks (2)
NH = LP // 512          # 512-col halves of the i axis (2)
NI = LP // 128          # i-chunks (8)
QW = 2 * LQ + D         # qside row width per batch (qt k0 | qt k1 | q_nat)

_NC_CACHE = None


def build_kernel():
    nc = bacc.Bacc("TRN2", debug=False, target_bir_lowering=False,
                   num_devices=NCORES)

    pt_in = nc.dram_tensor("pt", [128, BC, NK, LP], BF16,
                           kind="ExternalInput").ap()
    qs_in = nc.dram_tensor("qside", [128, BC, QW], BF16,
                           kind="ExternalInput").ap()
    qyb_in = nc.dram_tensor("qyb", [128, BC], F32, kind="ExternalInput").ap()
    pqt_out = nc.dram_tensor("pqt", [BC, NK, 128, LP], BF16,
                             kind="ExternalOutput").ap()
    u_out = nc.dram_tensor("u", [128, BC, NI], BF16, kind="ExternalOutput").ap()
    r_out = nc.dram_tensor("r", [128, BC, NI], F32, kind="ExternalOutput").ap()

    with tile.TileContext(nc) as tc:
        with tc.tile_pool(name="const", bufs=1) as const, \
             tc.tile_pool(name="sb", bufs=2) as sb, \
             tc.tile_pool(name="ps_st", bufs=2, space="PSUM") as ps_st, \
             tc.tile_pool(name="ps_en", bufs=2, space="PSUM") as ps_en, \
             tc.tile_pool(name="ps_y", bufs=2, space="PSUM") as ps_y:

            # ---- loads: batch 0 first, spread over the HWDGE queues; only
            # the last-needed pt goes on the slow SWDGE (gpsimd) queue ----
            qpk = const.tile([128, BC, QW], BF16)
            pt_all = const.tile([128, BC, NK, LP], BF16)
            qyb = const.tile([128, BC], F32)
            nc.sync.dma_start(out=qpk[:, 0], in_=qs_in[:, 0])
            nc.scalar.dma_start(out=pt_all[:, 0, 0], in_=pt_in[:, 0, 0])
            nc.sync.dma_start(out=pt_all[:, 0, 1], in_=pt_in[:, 0, 1])
            nc.scalar.dma_start(out=qyb[:], in_=qyb_in[:, :])
            nc.sync.dma_start(out=qpk[:, 1:], in_=qs_in[:, 1:])
            nc.scalar.dma_start(out=pt_all[:, 1], in_=pt_in[:, 1])
            nc.sync.dma_start(out=pt_all[:, 2], in_=pt_in[:, 2])
            nc.gpsimd.dma_start(out=pt_all[:, 3], in_=pt_in[:, 3])
            ident = const.tile([128, 128], BF16)
            make_identity(nc, ident[:])
            u_all = const.tile([128, BC, NI], BF16)
            r_all = const.tile([128, BC, NI], F32)

            def head(b):
                """S^T matmuls + exp -> eT (the j-partition layout of e)."""
                st = ps_st.tile([128, LP], F32, tag="st")
                for k in range(NK):
                    lhsT = qpk[:, b, k * LQ:(k + 1) * LQ]
                    for h in range(NH):
                        nc.tensor.matmul(
                            st[:, h * 512:(h + 1) * 512], lhsT=lhsT,
                            rhs=pt_all[:, b, k, h * 512:(h + 1) * 512],
                            start=(k == 0), stop=(k == NK - 1))
                eT = sb.tile([128, LP], BF16, tag="eT")
                for h in range(NH):
                    nc.scalar.activation(
                        eT[:, h * 512:(h + 1) * 512],
                        st[:, h * 512:(h + 1) * 512],
                        AF.Exp, bias=qyb[:, b:b + 1], scale=1.0)
                return eT

            def tail(b, eT):
                """Y^T + output store first (feeds the big DMA), then the
                u/r transposes + reduces (feed only tiny end DMAs)."""
                pqt_sb = sb.tile([128, NK, LP], BF16, tag="pqt_sb")
                for k in range(NK):
                    lhsT = qpk[:, b, 2 * LQ + k * 128:2 * LQ + (k + 1) * 128]
                    for h in range(NH):
                        yt = ps_y.tile([128, 512], F32, tag="yt")
                        nc.tensor.matmul(
                            yt[:], lhsT=lhsT,
                            rhs=eT[:, h * 512:(h + 1) * 512],
                            start=True, stop=True)
                        dst = pqt_sb[:, k, h * 512:(h + 1) * 512]
                        if h == 0:
                            nc.scalar.copy(dst, yt[:])
                        else:
                            nc.vector.tensor_copy(dst, yt[:])
                nc.sync.dma_start(out=pqt_out[b].rearrange("k p i -> p k i"),
                                  in_=pqt_sb[:])

                en3 = ps_en.tile([128, NI, 128], BF16, tag="en")
                for c in range(NI):
                    nc.tensor.transpose(en3[:, c, :],
                                        eT[:, c * 128:(c + 1) * 128], ident[:])
                nc.vector.reduce_max(out=u_all[:, b, :], in_=en3[:],
                                     axis=mybir.AxisListType.X)
                nc.vector.reduce_sum(out=r_all[:, b, :], in_=en3[:],
                                     axis=mybir.AxisListType.X)

            # software pipeline: S_{b+1} issues on PE before Y_b so the PE
            # queue has work while exp_b runs on ACT
            prev = None
            for b in range(BC):
                eT = head(b)
                if prev is not None:
                    tail(b - 1, prev)
                prev = eT
            tail(BC - 1, prev)

            nc.sync.dma_start(out=u_out[:, :, :], in_=u_all[:])
            nc.sync.dma_start(out=r_out[:, :, :], in_=r_all[:])

    nc.compile()
    return nc


def _get_nc():
    global _NC_CACHE
    if _NC_CACHE is None:
        _NC_CACHE = build_kernel()
    return _NC_CACHE


def _make_in_maps(paragraph, query, w, b):
    bf16 = ml_dtypes.bfloat16
    w = np.asarray(w, np.float32)
    wx, wy, wxy = w[:D], w[D:2 * D], w[2 * D:]

    p32 = np.asarray(paragraph, np.float32)
    q32 = np.asarray(query, np.float32)

    # qside, partition-major: [128, B, 2*Lq + D]
    qt_aug = ((q32 * wxy).transpose(0, 2, 1)
              + wx[None, :, None]).astype(bf16)               # [B, D, Lq]
    qside = np.empty((128, B, QW), bf16)
    qside[:, :, 0:LQ] = qt_aug[:, 0:128, :].transpose(1, 0, 2)
    qside[:, :, LQ:2 * LQ] = qt_aug[:, 128:256, :].transpose(1, 0, 2)
    qside[:, :, 2 * LQ:] = q32.astype(bf16).transpose(1, 0, 2)
    qyb = (q32 @ wy + np.float32(b)).astype(np.float32)       # [B, Lq]
    # pt, partition-major: [128, B, NK, LP]; pt[p, gb, k, i] = P[gb, i, 128k+p]
    pt = np.ascontiguousarray(
        p32.transpose(2, 0, 1).astype(bf16).reshape(NK, 128, B, LP)
        .transpose(1, 2, 0, 3))

    in_maps = []
    for m in range(NCORES):
        sl = slice(m * BC, (m + 1) * BC)
        in_maps.append({
            "pt": np.ascontiguousarray(pt[:, sl]),
            "qside": np.ascontiguousarray(qside[:, sl]),
            "qyb": np.ascontiguousarray(qyb[sl].T),
        })
    return in_maps, p32


def run(paragraph, query, w, b, trace=False, **trace_kwargs):
    """Compile (cached), execute on 8 cores, return ((pq, tiled_qp), results)."""
    nc = _get_nc()
    in_maps, p32 = _make_in_maps(paragraph, query, w, b)
    res = run_bass_kernel_spmd(nc, in_maps, core_ids=list(range(NCORES)),
                               trace=trace, **trace_kwargs)
    pqt = np.concatenate(
        [np.asarray(r["pqt"], np.float32) for r in res.results], axis=0)
    r_cols = np.concatenate(
        [np.asarray(r["r"], np.float32) for r in res.results], axis=1)
    r_ = r_cols.reshape(128, B, NI).transpose(1, 2, 0).reshape(B, LP)
    pq = (pqt.reshape(B, D, LP) / r_[:, None, :]).transpose(0, 2, 1)
    # u arrives as [128, BC, NI]: u[p, b, c] = max_j e at i = c*128 + p
    u_cols = np.concatenate(
        [np.asarray(r["u"], np.float32) for r in res.results], axis=1)
    u = u_cols.reshape(128, B, NI).transpose(1, 2, 0).reshape(B, LP)
    att = u / u.sum(axis=-1, keepdims=True)
    qp = (att[:, None, :] @ p32)[:, 0, :]                     # [B, D]
    tiled_qp = np.broadcast_to(qp[:, None, :], (B, LP, D))
    return (pq, tiled_qp), res


def kernel(paragraph, query, dm, qm, w, b):
    outs, _ = run(paragraph, query, w, b, trace=False)
    return outs


# revision 30
# speedup vs baseline: 1.1673x; 1.1673x over previous
"""AttentionFlow kernel for 8 TRN2 NeuronCores (Bass/Tile).

Math (per batch, masks are all-ones by problem spec):
    wx, wy, wxy = w[:D], w[D:2D], w[2D:]
    s[i,j]  = px[i] + qy[j] + sum_d P[i,d]*wxy[d]*Q[j,d] + b
    pq_att  = softmax_j(s);  pq[i,:] = sum_j pq_att[i,j] * Q[j,:]
    qp_sim  = max_j s;       qp_att = softmax_i(qp_sim)
    qp[:]   = sum_i qp_att[i] * P[i,:]   (tiled over Lp on host)

Device formulation (per core: BC=4 batches, data parallel over B):
    qt_aug[d,j] = wxy[d]*q[j,d] + wx[d]      (host; folds px into the S matmul:
                                              sum_d qt_aug[d,j]*pT[d,i] = s - qy[j] - b)
    S^T = qt_aug^T @ pT                       [j,i] in PSUM (bf16)
    e   = exp(S^T + qyb[j])                   (ACT, per-partition bias qyb = qy + b, host-made)
    u   = max_j e                             (PE 128x128 transposes + DVE free-dim reduce)
    r   = sum_j e                             (DVE free-dim reduce on the same en3)
    Y^T[d,i] = q_nat[:,d]^T @ e               (bf16 PE; copied out unnormalized in bf16)
Host post (cheap, ~0.2% of FLOPs): pq = (Y^T / r).T, qp = (u/sum u) @ P,
tiled broadcast. Softmax max-subtraction is skipped (|s| <= ~7, exp safe in
f32); ratios are mathematically identical to the reference.

Host prep: batch shards 4-per-core; bf16 casts; ALL device inputs are laid
out partition-major on the host so each DMA descriptor is one contiguous
multi-KB line per partition (small descriptors were the main startup cost).
Batch 0's operands go first on the fast HWDGE queues.
"""

import numpy as np
import ml_dtypes

import concourse.bass as bass
import concourse.mybir as mybir
import concourse.tile as tile
from concourse import bacc
from concourse.bass_utils import run_bass_kernel_spmd
from concourse.masks import make_identity

BF16 = mybir.dt.bfloat16
F32 = mybir.dt.float32
AF = mybir.ActivationFunctionType

B, LP, LQ, D = 32, 1024, 128, 256
NCORES = 8
BC = B // NCORES        # batches per core
NK = D // 128           # d-chunks (2)
NH = LP // 512          # 512-col halves of the i axis (2)
NI = LP // 128          # i-chunks (8)
QW = 2 * LQ + D         # qside row width per batch (qt k0 | qt k1 | q_nat)

_NC_CACHE = None


def build_kernel():
    nc = bacc.Bacc("TRN2", debug=False, target_bir_lowering=False,
                   num_devices=NCORES)

    pt_in = nc.dram_tensor("pt", [128, BC, NK, LP], BF16,
                           kind="ExternalInput").ap()
    qs_in = nc.dram_tensor("qside", [128, BC, QW], BF16,
                           kind="ExternalInput").ap()
    qyb_in = nc.dram_tensor("qyb", [128, BC], F32, kind="ExternalInput").ap()
    pqt_out = nc.dram_tensor("pqt", [BC, NK, 128, LP], BF16,
                             kind="ExternalOutput").ap()
    u_out = nc.dram_tensor("u", [128, BC, NI], BF16, kind="ExternalOutput").ap()
    r_out = nc.dram_tensor("r", [128, BC, NI], F32, kind="ExternalOutput").ap()

    with tile.TileContext(nc) as tc:
        with tc.tile_pool(name="const", bufs=1) as const, \
             tc.tile_pool(name="sb", bufs=2) as sb, \
             tc.tile_pool(name="ps_st", bufs=2, space="PSUM") as ps_st, \
             tc.tile_pool(name="ps_en", bufs=2, space="PSUM") as ps_en, \
             tc.tile_pool(name="ps_y", bufs=2, space="PSUM") as ps_y:

            # ---- loads: batch 0 first, spread over the HWDGE queues; only
            # the last-needed pt goes on the slow SWDGE (gpsimd) queue ----
            qpk = const.tile([128, BC, QW], BF16)
            pt_all = const.tile([128, BC, NK, LP], BF16)
            qyb = const.tile([128, BC], F32)
            nc.sync.dma_start(out=qpk[:, 0], in_=qs_in[:, 0])
            nc.scalar.dma_start(out=pt_all[:, 0, 0], in_=pt_in[:, 0, 0])
            nc.sync.dma_start(out=pt_all[:, 0, 1], in_=pt_in[:, 0, 1])
            nc.scalar.dma_start(out=qyb[:], in_=qyb_in[:, :])
            nc.sync.dma_start(out=qpk[:, 1:], in_=qs_in[:, 1:])
            nc.scalar.dma_start(out=pt_all[:, 1], in_=pt_in[:, 1])
            nc.sync.dma_start(out=pt_all[:, 2], in_=pt_in[:, 2])
            nc.gpsimd.dma_start(out=pt_all[:, 3], in_=pt_in[:, 3])
            ident = const.tile([128, 128], BF16)
            make_identity(nc, ident[:])
            u_all = const.tile([128, BC, NI], BF16)
            r_all = const.tile([128, BC, NI], F32)

            def head(b):
                """S^T matmuls + exp -> eT (the j-partition layout of e)."""
                st = ps_st.tile([128, LP], F32, tag="st")
                for k in range(NK):
                    lhsT = qpk[:, b, k * LQ:(k + 1) * LQ]
                    for h in range(NH):
                        nc.tensor.matmul(
                            st[:, h * 512:(h + 1) * 512], lhsT=lhsT,
                            rhs=pt_all[:, b, k, h * 512:(h + 1) * 512],
                            start=(k == 0), stop=(k == NK - 1))
                eT = sb.tile([128, LP], BF16, tag="eT")
                for h in range(NH):
                    nc.scalar.activation(
                        eT[:, h * 512:(h + 1) * 512],
                        st[:, h * 512:(h + 1) * 512],
                        AF.Exp, bias=qyb[:, b:b + 1], scale=1.0)
                return eT

            def tail(b, eT):
                """u/r via PE transposes + DVE reduces; Y^T; stores."""
                en3 = ps_en.tile([128, NI, 128], BF16, tag="en")
                for c in range(NI):
                    nc.tensor.transpose(en3[:, c, :],
                                        eT[:, c * 128:(c + 1) * 128], ident[:])
                nc.vector.reduce_max(out=u_all[:, b, :], in_=en3[:],
                                     axis=mybir.AxisListType.X)
                nc.vector.reduce_sum(out=r_all[:, b, :], in_=en3[:],
                                     axis=mybir.AxisListType.X)

                # Y^T = q_nat^T @ e, drained unnormalized as bf16
                pqt_sb = sb.tile([128, NK, LP], BF16, tag="pqt_sb")
                for k in range(NK):
                    lhsT = qpk[:, b, 2 * LQ + k * 128:2 * LQ + (k + 1) * 128]
                    for h in range(NH):
                        yt = ps_y.tile([128, 512], F32, tag="yt")
                        nc.tensor.matmul(
                            yt[:], lhsT=lhsT,
                            rhs=eT[:, h * 512:(h + 1) * 512],
                            start=True, stop=True)
                        dst = pqt_sb[:, k, h * 512:(h + 1) * 512]
                        if h == 0:
                            nc.scalar.copy(dst, yt[:])
                        else:
                            nc.vector.tensor_copy(dst, yt[:])
                nc.sync.dma_start(out=pqt_out[b].rearrange("k p i -> p k i"),
                                  in_=pqt_sb[:])

            # software pipeline: S_{b+1} issues on PE before Y_b so the PE
            # queue has work while exp_b runs on ACT
            prev = None
            for b in range(BC):
                eT = head(b)
                if prev is not None:
                    tail(b - 1, prev)
                prev = eT
            tail(BC - 1, prev)

            nc.sync.dma_start(out=u_out[:, :, :], in_=u_all[:])
            nc.sync.dma_start(out=r_out[:, :, :], in_=r_all[:])

    nc.compile()
    return nc


def _get_nc():
    global _NC_CACHE
    if _NC_CACHE is None:
        _NC_CACHE = build_kernel()
    return _NC_CACHE


def _make_in_maps(paragraph, query, w, b):
    bf16 = ml_dtypes.bfloat16
    w = np.asarray(w, np.float32)
    wx, wy, wxy = w[:D], w[D:2 * D], w[2 * D:]

    p32 = np.asarray(paragraph, np.float32)
    q32 = np.asarray(query, np.float32)

    # qside, partition-major: [128, B, 2*Lq + D]
    qt_aug = ((q32 * wxy).transpose(0, 2, 1)
              + wx[None, :, None]).astype(bf16)               # [B, D, Lq]
    qside = np.empty((128, B, QW), bf16)
    qside[:, :, 0:LQ] = qt_aug[:, 0:128, :].transpose(1, 0, 2)
    qside[:, :, LQ:2 * LQ] = qt_aug[:, 128:256, :].transpose(1, 0, 2)
    qside[:, :, 2 * LQ:] = q32.astype(bf16).transpose(1, 0, 2)
    qyb = (q32 @ wy + np.float32(b)).astype(np.float32)       # [B, Lq]
    # pt, partition-major: [128, B, NK, LP]; pt[p, gb, k, i] = P[gb, i, 128k+p]
    pt = np.ascontiguousarray(
        p32.transpose(2, 0, 1).astype(bf16).reshape(NK, 128, B, LP)
        .transpose(1, 2, 0, 3))

    in_maps = []
    for m in range(NCORES):
        sl = slice(m * BC, (m + 1) * BC)
        in_maps.append({
            "pt": np.ascontiguousarray(pt[:, sl]),
            "qside": np.ascontiguousarray(qside[:, sl]),
            "qyb": np.ascontiguousarray(qyb[sl].T),
        })
    return in_maps, p32


def run(paragraph, query, w, b, trace=False, **trace_kwargs):
    """Compile (cached), execute on 8 cores, return ((pq, tiled_qp), results)."""
    nc = _get_nc()
    in_maps, p32 = _make_in_maps(paragraph, query, w, b)
    res = run_bass_kernel_spmd(nc, in_maps, core_ids=list(range(NCORES)),
                               trace=trace, **trace_kwargs)
    pqt = np.concatenate(
        [np.asarray(r["pqt"], np.float32) for r in res.results], axis=0)
    r_cols = np.concatenate(
        [np.asarray(r["r"], np.float32) for r in res.results], axis=1)
    r_ = r_cols.reshape(128, B, NI).transpose(1, 2, 0).reshape(B, LP)
    pq = (pqt.reshape(B, D, LP) / r_[:, None, :]).transpose(0, 2, 1)
    # u arrives as [128, BC, NI]: u[p, b, c] = max_j e at i = c*128 + p
    u_cols = np.concatenate(
        [np.asarray(r["u"], np.float32) for r in res.results], axis=1)
    u = u_cols.reshape(128, B, NI).transpose(1, 2, 0).reshape(B, LP)
    att = u / u.sum(axis=-1, keepdims=True)
    qp = (att[:, None, :] @ p32)[:, 0, :]                     # [B, D]
    tiled_qp = np.broadcast_to(qp[:, None, :], (B, LP, D))
    return (pq, tiled_qp), res


def kernel(paragraph, query, dm, qm, w, b):
    outs, _ = run(paragraph, query, w, b, trace=False)
    return outs
